# revision 21
# baseline (speedup 1.0000x reference)
"""Trainium2 Bass kernel: batched single-head attention + gate MLP.

Per-core (data-parallel over batch, 1 batch row per core):
  q = query @ Wq.T + bq ; k,v likewise
  scores = q @ k.T / sqrt(768); attn = softmax(scores)
  attended = attn @ v
  h = relu(attended @ Wg1.T + bg1); gate = sigmoid(h @ Wg2.T + bg2)
  out = sigmoid(gate) * attended * text_scale

Weights arrive pre-transposed from the host ([d, e] layout) so only the
three activation inputs are transposed on the PE. q is projected on
demand into a 3-slot SBUF ring inside the attention loop (no qT in
DRAM). v and the exp'd scores are stored bf16; the normalized attended
is evicted twice (bf16 for the gate matmul, f32r for the output path)
so the output is never quantized below f32r. The v bias is folded into
bg1 on the host plus a fused (att+bv)*(ts/2) op on the Pool engine,
legal because softmax rows sum to 1. Sigmoids use the tanh half-angle
identity so every activation lives in one act-function table set
(exp_and_others) — a single table load for the whole kernel.

Scheduling: one software-pipelined chunk loop (transpose chunk i+2
after projecting chunk i) keeps the PE fed through the projections;
in the attention loop the previous iteration's gate tail + output
transposes are emitted between attended and gate1, and the last
iteration runs a per-block tail to shorten the drain.
"""
import numpy as np
import ml_dtypes

import concourse.bass as bass
import concourse.mybir as mybir
import concourse.tile as tile
from concourse import bacc
from concourse.bass_utils import run_bass_kernel_spmd

F32 = mybir.dt.float32
F32R = mybir.dt.float32r
BF16 = mybir.dt.bfloat16
AF = mybir.ActivationFunctionType
ALU = mybir.AluOpType

B, S, D = 8, 2048, 768
EB = D // 128            # 6 feature blocks
SB = S // 128            # 16 seq blocks
CH = 256                 # seq chunk = attention i-chunk
NCH = S // CH            # 8
SCALE = 1.0 / float(np.sqrt(D))

_CACHE = {}


def _build():
    nc = bacc.Bacc(None)

    query = nc.dram_tensor("query", [S, D], F32R, kind="ExternalInput")
    key = nc.dram_tensor("key", [S, D], F32R, kind="ExternalInput")
    value = nc.dram_tensor("value", [S, D], F32R, kind="ExternalInput")
    wqT = nc.dram_tensor("wqT", [D, D], F32R, kind="ExternalInput")
    wkT = nc.dram_tensor("wkT", [D, D], F32R, kind="ExternalInput")
    wvT = nc.dram_tensor("wvT", [D, D], F32R, kind="ExternalInput")
    wg1T = nc.dram_tensor("wg1T", [D, D], BF16, kind="ExternalInput")
    wg2T = nc.dram_tensor("wg2T", [D, D], BF16, kind="ExternalInput")
    bq = nc.dram_tensor("bq", [D], F32, kind="ExternalInput")
    bk = nc.dram_tensor("bk", [D], F32, kind="ExternalInput")
    bv = nc.dram_tensor("bv", [D], F32, kind="ExternalInput")
    bg1a = nc.dram_tensor("bg1a", [D], F32, kind="ExternalInput")
    bg2 = nc.dram_tensor("bg2", [D], F32, kind="ExternalInput")
    ts = nc.dram_tensor("ts", [1, D], F32, kind="ExternalInput")
    ident = nc.dram_tensor("ident", [128, 128], F32R, kind="ExternalInput")
    ones = nc.dram_tensor("ones", [128, 128], BF16, kind="ExternalInput")
    out = nc.dram_tensor("out", [S, D], F32, kind="ExternalOutput")

    with tile.TileContext(nc) as tc:
        with tc.tile_pool(name="persist", bufs=1) as P, \
             tc.tile_pool(name="psc", bufs=4, space="PSUM") as PSC, \
             tc.tile_pool(name="pmm", bufs=1, space="PSUM") as PMM, \
             tc.tile_pool(name="pdn", bufs=1, space="PSUM") as PDN, \
             tc.tile_pool(name="ppg", bufs=2, space="PSUM") as PPG, \
             tc.tile_pool(name="abq", bufs=1) as ABQ:

            ident_sb = P.tile([128, 128], F32R, tag="ident")
            nc.sync.dma_start(out=ident_sb, in_=ident[:, :])
            c25_sb = P.tile([128, 1], F32, tag="c25")
            nc.vector.memset(c25_sb, 0.25)

            def vec_sb(name, src):                       # [D] -> [128, EB]
                t = P.tile([128, EB], F32, tag=name, name=name)
                nc.sync.dma_start(out=t, in_=src.rearrange("(b p) -> p b", p=128))
                return t

            kT = P.tile([128, EB, S], F32R, tag="kT")        # k^T [e, s]
            v_sb = P.tile([128, SB, D], BF16, tag="v")       # v [j, e]
            wg1_sb = P.tile([128, EB, D], BF16, tag="wg1")
            wg2_sb = P.tile([128, EB, D], BF16, tag="wg2")

            wq_sb = ABQ.tile([128, EB, D], F32R, tag="wq")

            def load_w(dst, wdram):
                nc.sync.dma_start(
                    out=dst, in_=wdram.rearrange("(db p) e -> p db e", p=128))

            # ---- staged input pipeline (key 0..7, value 8..15, query 16..23)
            order = [(key, c) for c in range(NCH)] + \
                    [(value, c) for c in range(NCH)] + \
                    [(query, c) for c in range(NCH)]
            xsts = {}
            xTs = {}

            def stage_idx(i):
                src, c = order[i]
                xst = ABQ.tile([128, 2, D], F32R, tag="xst", bufs=2)
                nc.sync.dma_start(
                    out=xst,
                    in_=src[c * CH:(c + 1) * CH, :].rearrange(
                        "(sb p) d -> p sb d", p=128))
                xsts[i] = xst

            def trans_chunk(i):
                """PE-transpose staged chunk i -> xT [d-part, db, s]."""
                xst = xsts.pop(i)
                if i + 2 < len(order):
                    stage_idx(i + 2)
                xT = ABQ.tile([128, EB, CH], F32R, tag="xT", bufs=2)
                n = 0
                for sb in range(2):
                    for db0 in (0, 3):
                        tp = PSC.tile([128, 3, 128], F32R, tag="sc")
                        for k3 in range(3):
                            nc.tensor.transpose(
                                tp[:, k3, :],
                                xst[:, sb, (db0 + k3) * 128:(db0 + k3 + 1) * 128],
                                ident_sb)
                        dst = xT[:, db0:db0 + 3, sb * 128:(sb + 1) * 128]
                        if n == 1:
                            nc.scalar.copy(dst, tp)
                        else:
                            nc.vector.tensor_copy(dst, tp)
                        n += 1
                xTs[i] = xT

            def proj_T(xT, w_sb, dst, bias_sb):
                """Transposed projection: dst[:, eb, :] = (W x^T + b)[e-blk, i]."""
                for eb in range(EB):
                    mmt = PMM.tile([128, CH], F32, tag="mm")
                    for db in range(EB):
                        nc.tensor.matmul(
                            mmt, w_sb[:, db, eb * 128:(eb + 1) * 128], xT[:, db, :],
                            start=(db == 0), stop=(db == EB - 1))
                    nc.scalar.activation(
                        dst[:, eb, :], mmt, AF.Identity, bias=bias_sb[:, eb:eb + 1])

            def proj_v(xT, w_sb, c):
                """Natural projection: v[j, e] blocks, no bias (folded out)."""
                for jbh in range(2):
                    for h, (n0, n1) in enumerate(((0, 384), (384, 768))):
                        mmt = PMM.tile([128, 384], F32, tag="mm")
                        for db in range(EB):
                            nc.tensor.matmul(
                                mmt, xT[:, db, jbh * 128:(jbh + 1) * 128],
                                w_sb[:, db, n0:n1],
                                start=(db == 0), stop=(db == EB - 1))
                        if h == 0:
                            nc.vector.tensor_copy(v_sb[:, c * 2 + jbh, n0:n1], mmt)
                        else:
                            nc.scalar.copy(v_sb[:, c * 2 + jbh, n0:n1], mmt)

            qbufs = [None] * NCH

            # ---- phase AB: project key and value, then first two q chunks
            with tc.tile_pool(name="abkv", bufs=1) as ABKV:
                wk_sb = ABKV.tile([128, EB, D], F32R, tag="wk")
                wv_sb = ABKV.tile([128, EB, D], F32R, tag="wv")
                stage_idx(0)
                stage_idx(1)
                # wk in halves so the first projection chain can start on
                # the first half while the second transfers
                nc.sync.dma_start(
                    out=wk_sb[:, 0:3, :],
                    in_=wkT[0:384, :].rearrange("(db p) e -> p db e", p=128))
                nc.sync.dma_start(
                    out=wk_sb[:, 3:6, :],
                    in_=wkT[384:768, :].rearrange("(db p) e -> p db e", p=128))
                # small constants after the critical-path loads
                ones_sb = P.tile([128, 128], BF16, tag="ones")
                nc.sync.dma_start(out=ones_sb, in_=ones[:, :])
                bq_sb = vec_sb("bq", bq[:])
                bk_sb = vec_sb("bk", bk[:])
                bv_sb = vec_sb("bv", bv[:])
                bg1_sb = vec_sb("bg1", bg1a[:])
                bg2_sb = vec_sb("bg2", bg2[:])      # host passes bg2/2
                ts_sb = vec_sb("ts", ts[0, :])      # host passes ts/2

                def proj_idx(i):
                    xT = xTs.pop(i)
                    kind, c = divmod(i, NCH)
                    if kind == 0:
                        proj_T(xT, wk_sb, kT[:, :, c * CH:(c + 1) * CH], bk_sb)
                    elif kind == 1:
                        proj_v(xT, wv_sb, c)
                    else:
                        qb = P.tile([128, EB, CH], F32R, tag="qbuf", bufs=3,
                                    name=f"qbuf{c}")
                        proj_T(xT, wq_sb, qb, bq_sb)
                        qbufs[c] = qb

                trans_chunk(0)
                trans_chunk(1)
                for i in range(2 * NCH + 2):         # key, value, q0, q1
                    proj_idx(i)
                    if i + 2 < 2 * NCH + 4:          # transposes up to q3
                        trans_chunk(i + 2)
                    if i == 3:
                        load_w(wv_sb, wvT)
                    elif i == 11:
                        load_w(wq_sb, wqT)
                    elif i == 14:
                        load_w(wg1_sb, wg1T)
                    elif i == 15:
                        load_w(wg2_sb, wg2T)

            # ---- phase C: attention + gate, software-pipelined over i-chunks
            with tc.tile_pool(name="phc", bufs=1) as CP:
                attnT = CP.tile([128, SB, CH], BF16, tag="attnT")
                attTb = CP.tile([128, EB * CH], BF16, tag="attTb")   # gate path
                attTf = CP.tile([128, EB * CH], F32R, tag="attTf")   # output path
                hT = CP.tile([128, EB * CH], BF16, tag="hT")
                g2 = CP.tile([128, EB * CH], BF16, tag="g2")         # tanh(gate/2)
                g3 = CP.tile([128, EB * CH], F32, tag="g3")          # tanh(s1/2)
                gated = CP.tile([128, EB * CH], F32R, tag="gated")
                avs = [None] * NCH            # (att+bv)*ts/2, double-buffered
                                              # across iterations for the tail

                def tail_math(j, sl):
                    """out = (1 + tanh(.25*g2 + .25)) * av on a column slice.

                    s1 = sigmoid(gate) = .5 + .5*g2 ; s2 = sigmoid(s1)
                    s2*att*ts = (1 + tanh(.25*g2 + .25)) * (att+bv)*ts/2
                    """
                    nc.scalar.activation(
                        g3[:, sl], g2[:, sl], AF.Tanh, bias=c25_sb, scale=0.25)
                    nc.vector.scalar_tensor_tensor(
                        gated[:, sl], g3[:, sl], 1.0, avs[j][:, sl],
                        ALU.add, ALU.mult)

                def tail_out(j):
                    """output transposes + store for iteration j (half-major
                    so the last iteration's first halves drain early)."""
                    gv = gated.rearrange("p (eb i) -> p eb i", eb=EB)
                    osbs = [CP.tile([128, D], F32, tag="osb", bufs=2,
                                    name="osb") for _ in range(2)]
                    for half in range(2):
                        cs = slice(half * 384, (half + 1) * 384)
                        for ib in range(2):
                            po = PSC.tile([128, 3, 128], F32R, tag="sc")
                            for k3 in range(3):
                                eb = half * 3 + k3
                                nc.tensor.transpose(
                                    po[:, k3, :],
                                    gv[:, eb, ib * 128:(ib + 1) * 128], ident_sb)
                            nc.vector.tensor_copy(osbs[ib][:, cs], po)
                            r0 = (j * 2 + ib) * 128
                            nc.sync.dma_start(
                                out=out[r0:r0 + 128, cs], in_=osbs[ib][:, cs])

                for ic in range(NCH):
                    qb = qbufs[ic]
                    last = ic == NCH - 1
                    # scores^T + exp per j-block, with the softmax-denominator
                    # chain (ones^T @ exp) interleaved one block behind so the
                    # PE never waits on the last exp
                    dn = PDN.tile([128, CH], F32, tag="dn")
                    for jb in range(SB):
                        ps = PSC.tile([128, CH], F32, tag="sc")
                        for eb in range(EB):
                            nc.tensor.matmul(
                                ps, kT[:, eb, jb * 128:(jb + 1) * 128],
                                qb[:, eb, :],
                                start=(eb == 0), stop=(eb == EB - 1))
                        nc.scalar.activation(
                            attnT[:, jb, :], ps, AF.Exp, scale=SCALE)
                        if jb >= 2:
                            nc.tensor.matmul(
                                dn, ones_sb, attnT[:, jb - 2, :],
                                start=(jb == 2), stop=False)
                    for jb in (SB - 2, SB - 1):
                        nc.tensor.matmul(
                            dn, ones_sb, attnT[:, jb, :],
                            start=False, stop=(jb == SB - 1))
                    recip = CP.tile([128, CH], F32, tag="recip", bufs=1,
                                    name="recip")
                    nc.vector.reciprocal(recip, dn)
                    # next q chunk rides behind the scores
                    if ic + 2 < NCH:
                        proj_idx(2 * NCH + ic + 2)
                        if 2 * NCH + ic + 4 < len(order):
                            trans_chunk(2 * NCH + ic + 4)
                    # attended^T; normalize on DVE; (att+bv)*ts/2 on Pool
                    av = CP.tile([128, EB * CH], F32R, tag="av", bufs=2,
                                 name=f"av{ic}")
                    avs[ic] = av
                    for eb in range(EB):
                        pa = PMM.tile([128, CH], F32, tag="mm")
                        for jb in range(SB):
                            nc.tensor.matmul(
                                pa, v_sb[:, jb, eb * 128:(eb + 1) * 128],
                                attnT[:, jb, :],
                                start=(jb == 0), stop=(jb == SB - 1))
                        sl = slice(eb * CH, (eb + 1) * CH)
                        nc.vector.tensor_mul(attTb[:, sl], pa, recip)
                        nc.vector.tensor_mul(attTf[:, sl], pa, recip)
                        nc.gpsimd.tensor_scalar(
                            av[:, sl], attTf[:, sl], bv_sb[:, eb:eb + 1],
                            ts_sb[:, eb:eb + 1], ALU.add, ALU.mult)
                    # previous iteration's tail fills the attT-eviction wait
                    if ic > 0:
                        tail_math(ic - 1, slice(0, EB * CH))
                        tail_out(ic - 1)
                    # gate1: h = relu(Wg1 att + bg1')
                    for e2 in range(EB):
                        ph = PPG.tile([128, CH], F32, tag="pg")
                        for eb in range(EB):
                            nc.tensor.matmul(
                                ph, wg1_sb[:, eb, e2 * 128:(e2 + 1) * 128],
                                attTb[:, eb * CH:(eb + 1) * CH],
                                start=(eb == 0), stop=(eb == EB - 1))
                        nc.scalar.activation(
                            hT[:, e2 * CH:(e2 + 1) * CH], ph, AF.Relu,
                            bias=bg1_sb[:, e2:e2 + 1])
                    # gate2: g2 = tanh((Wg2 h + bg2)/2); fine-grained tail on
                    # the last iteration so the drain chain is short
                    for e2 in range(EB):
                        pg = PPG.tile([128, CH], F32, tag="pg")
                        for eb in range(EB):
                            nc.tensor.matmul(
                                pg, wg2_sb[:, eb, e2 * 128:(e2 + 1) * 128],
                                hT[:, eb * CH:(eb + 1) * CH],
                                start=(eb == 0), stop=(eb == EB - 1))
                        sl = slice(e2 * CH, (e2 + 1) * CH)
                        nc.scalar.activation(
                            g2[:, sl], pg, AF.Tanh,
                            bias=bg2_sb[:, e2:e2 + 1], scale=0.5)
                        if last:
                            tail_math(ic, sl)
                if NCH:
                    tail_out(NCH - 1)

    nc.compile()
    return nc


def kernel(**inputs):
    if "nc" not in _CACHE:
        _CACHE["nc"] = _build()
    nc = _CACHE["nc"]
    q = np.ascontiguousarray(inputs["query"], dtype=np.float32)
    k = np.ascontiguousarray(inputs["key"], dtype=np.float32)
    vv = np.ascontiguousarray(inputs["value"], dtype=np.float32)
    Wg1 = np.asarray(inputs["Wg1"], np.float32)
    bv_np = np.asarray(inputs["bv"], np.float32)
    bg1a = np.asarray(inputs["bg1"], np.float32) + Wg1 @ bv_np
    shared = {
        "wqT": np.ascontiguousarray(np.asarray(inputs["Wq"], np.float32).T),
        "wkT": np.ascontiguousarray(np.asarray(inputs["Wk"], np.float32).T),
        "wvT": np.ascontiguousarray(np.asarray(inputs["Wv"], np.float32).T),
        "wg1T": np.ascontiguousarray(
            Wg1.T.astype(ml_dtypes.bfloat16)),
        "wg2T": np.ascontiguousarray(
            np.asarray(inputs["Wg2"], np.float32).T.astype(ml_dtypes.bfloat16)),
        "bq": np.ascontiguousarray(inputs["bq"], np.float32),
        "bk": np.ascontiguousarray(inputs["bk"], np.float32),
        "bv": np.ascontiguousarray(bv_np),
        "bg1a": np.ascontiguousarray(bg1a),
        "bg2": np.ascontiguousarray(
            np.asarray(inputs["bg2"], np.float32) * 0.5),
        "ts": np.ascontiguousarray(
            np.asarray(inputs["text_scale"], np.float32) * 0.5),
        "ident": np.eye(128, dtype=np.float32),
        "ones": np.ones((128, 128), dtype=ml_dtypes.bfloat16),
    }
    in_maps = [
        dict(shared, query=q[b], key=k[b], value=vv[b]) for b in range(B)
    ]
    trace = bool(inputs.get("_trace"))
    r = run_bass_kernel_spmd(nc, in_maps, list(range(B)), trace=trace)
    if trace:
        print("HW exec time:", r.exec_time_ns, "ns")
        _CACHE["last_result"] = r
    return np.stack([r.results[b]["out"] for b in range(B)], axis=0)


if __name__ == "__main__":
    pass


# revision 22
# speedup vs baseline: 1.2116x; 1.2116x over previous
"""Trainium2 Bass kernel: batched single-head attention + gate MLP.

Per-core (data-parallel over batch, 1 batch row per core):
  q = query @ Wq.T + bq ; k,v likewise
  scores = q @ k.T / sqrt(768); attn = softmax(scores)
  attended = attn @ v
  h = relu(attended @ Wg1.T + bg1); gate = sigmoid(h @ Wg2.T + bg2)
  out = sigmoid(gate) * attended * text_scale

Weights arrive pre-transposed from the host ([d, e] layout) so only the
three activation inputs are transposed on the PE. q is projected on
demand into a 3-slot SBUF ring inside the attention loop (no qT in
DRAM). v and the exp'd scores are stored bf16; the normalized attended
is evicted twice (bf16 for the gate matmul, f32r for the output path)
so the output is never quantized below f32r. The v bias is folded into
bg1 on the host plus a fused (att+bv)*(ts/2) op on the Pool engine,
legal because softmax rows sum to 1. Sigmoids use the tanh half-angle
identity so every activation lives in one act-function table set
(exp_and_others) — a single table load for the whole kernel.

Scheduling: one software-pipelined chunk loop (transpose chunk i+2
after projecting chunk i) keeps the PE fed through the projections;
in the attention loop the previous iteration's gate tail + output
transposes are emitted between attended and gate1, and the last
iteration runs a per-block tail to shorten the drain.
"""
import numpy as np
import ml_dtypes

import concourse.bass as bass
import concourse.mybir as mybir
import concourse.tile as tile
from concourse import bacc
from concourse.bass_utils import run_bass_kernel_spmd

F32 = mybir.dt.float32
F32R = mybir.dt.float32r
BF16 = mybir.dt.bfloat16
AF = mybir.ActivationFunctionType
ALU = mybir.AluOpType

B, S, D = 8, 2048, 768
EB = D // 128            # 6 feature blocks
SB = S // 128            # 16 seq blocks
CH = 256                 # seq chunk = attention i-chunk
NCH = S // CH            # 8
SCALE = 1.0 / float(np.sqrt(D))

_CACHE = {}


def _build():
    nc = bacc.Bacc(None)

    query = nc.dram_tensor("query", [S, D], F32R, kind="ExternalInput")
    key = nc.dram_tensor("key", [S, D], F32R, kind="ExternalInput")
    value = nc.dram_tensor("value", [S, D], F32R, kind="ExternalInput")
    wqT = nc.dram_tensor("wqT", [D, D], F32R, kind="ExternalInput")
    wkT = nc.dram_tensor("wkT", [D, D], F32R, kind="ExternalInput")
    wvT = nc.dram_tensor("wvT", [D, D], F32R, kind="ExternalInput")
    wg1T = nc.dram_tensor("wg1T", [D, D], BF16, kind="ExternalInput")
    wg2T = nc.dram_tensor("wg2T", [D, D], BF16, kind="ExternalInput")
    bq = nc.dram_tensor("bq", [D], F32, kind="ExternalInput")
    bk = nc.dram_tensor("bk", [D], F32, kind="ExternalInput")
    bv = nc.dram_tensor("bv", [D], F32, kind="ExternalInput")
    bg1a = nc.dram_tensor("bg1a", [D], F32, kind="ExternalInput")
    bg2 = nc.dram_tensor("bg2", [D], F32, kind="ExternalInput")
    ts = nc.dram_tensor("ts", [1, D], F32, kind="ExternalInput")
    ident = nc.dram_tensor("ident", [128, 128], F32R, kind="ExternalInput")
    ones = nc.dram_tensor("ones", [128, 128], BF16, kind="ExternalInput")
    out = nc.dram_tensor("out", [S, D], F32, kind="ExternalOutput")

    with tile.TileContext(nc) as tc:
        with tc.tile_pool(name="persist", bufs=1) as P, \
             tc.tile_pool(name="psc", bufs=3, space="PSUM") as PSC, \
             tc.tile_pool(name="pmm", bufs=2, space="PSUM") as PMM, \
             tc.tile_pool(name="pdn", bufs=1, space="PSUM") as PDN, \
             tc.tile_pool(name="ppg", bufs=2, space="PSUM") as PPG, \
             tc.tile_pool(name="abq", bufs=1) as ABQ:

            ident_sb = P.tile([128, 128], F32R, tag="ident")
            nc.sync.dma_start(out=ident_sb, in_=ident[:, :])
            c25_sb = P.tile([128, 1], F32, tag="c25")
            nc.vector.memset(c25_sb, 0.25)

            def vec_sb(name, src):                       # [D] -> [128, EB]
                t = P.tile([128, EB], F32, tag=name, name=name)
                nc.sync.dma_start(out=t, in_=src.rearrange("(b p) -> p b", p=128))
                return t

            kT = P.tile([128, EB, S], F32R, tag="kT")        # k^T [e, s]
            v_sb = P.tile([128, SB, D], BF16, tag="v")       # v [j, e]
            wg1_sb = P.tile([128, EB, D], BF16, tag="wg1")
            wg2_sb = P.tile([128, EB, D], BF16, tag="wg2")

            wq_sb = ABQ.tile([128, EB, D], F32R, tag="wq")

            def load_w(dst, wdram):
                nc.sync.dma_start(
                    out=dst, in_=wdram.rearrange("(db p) e -> p db e", p=128))

            # ---- staged input pipeline (key 0..7, value 8..15, query 16..23)
            order = [(key, c) for c in range(NCH)] + \
                    [(value, c) for c in range(NCH)] + \
                    [(query, c) for c in range(NCH)]
            xsts = {}
            xTs = {}

            def stage_idx(i):
                src, c = order[i]
                xst = ABQ.tile([128, 2, D], F32R, tag="xst", bufs=2)
                nc.sync.dma_start(
                    out=xst,
                    in_=src[c * CH:(c + 1) * CH, :].rearrange(
                        "(sb p) d -> p sb d", p=128))
                xsts[i] = xst

            def trans_chunk(i):
                """PE-transpose staged chunk i -> xT [d-part, db, s]."""
                xst = xsts.pop(i)
                if i + 2 < len(order):
                    stage_idx(i + 2)
                xT = ABQ.tile([128, EB, CH], F32R, tag="xT", bufs=2)
                n = 0
                for sb in range(2):
                    for db0 in (0, 3):
                        tp = PSC.tile([128, 3, 128], F32R, tag="sc")
                        for k3 in range(3):
                            nc.tensor.transpose(
                                tp[:, k3, :],
                                xst[:, sb, (db0 + k3) * 128:(db0 + k3 + 1) * 128],
                                ident_sb)
                        dst = xT[:, db0:db0 + 3, sb * 128:(sb + 1) * 128]
                        if n == 1:
                            nc.scalar.copy(dst, tp)
                        else:
                            nc.vector.tensor_copy(dst, tp)
                        n += 1
                xTs[i] = xT

            def proj_T(xT, w_sb, dst, bias_sb):
                """Transposed projection: dst[:, eb, :] = (W x^T + b)[e-blk, i]."""
                for eb in range(EB):
                    mmt = PMM.tile([128, CH], F32, tag="mm")
                    for db in range(EB):
                        nc.tensor.matmul(
                            mmt, w_sb[:, db, eb * 128:(eb + 1) * 128], xT[:, db, :],
                            start=(db == 0), stop=(db == EB - 1))
                    nc.scalar.activation(
                        dst[:, eb, :], mmt, AF.Identity, bias=bias_sb[:, eb:eb + 1])

            def proj_v(xT, w_sb, c):
                """Natural projection: v[j, e] blocks, no bias (folded out)."""
                for jbh in range(2):
                    for h, (n0, n1) in enumerate(((0, 384), (384, 768))):
                        mmt = PMM.tile([128, 384], F32, tag="mm")
                        for db in range(EB):
                            nc.tensor.matmul(
                                mmt, xT[:, db, jbh * 128:(jbh + 1) * 128],
                                w_sb[:, db, n0:n1],
                                start=(db == 0), stop=(db == EB - 1))
                        if h == 0:
                            nc.vector.tensor_copy(v_sb[:, c * 2 + jbh, n0:n1], mmt)
                        else:
                            nc.scalar.copy(v_sb[:, c * 2 + jbh, n0:n1], mmt)

            qbufs = [None] * NCH

            # ---- phase AB: project key and value, then first two q chunks
            with tc.tile_pool(name="abkv", bufs=1) as ABKV:
                wk_sb = ABKV.tile([128, EB, D], F32R, tag="wk")
                wv_sb = ABKV.tile([128, EB, D], F32R, tag="wv")
                stage_idx(0)
                stage_idx(1)
                # wk in halves so the first projection chain can start on
                # the first half while the second transfers
                nc.sync.dma_start(
                    out=wk_sb[:, 0:3, :],
                    in_=wkT[0:384, :].rearrange("(db p) e -> p db e", p=128))
                nc.sync.dma_start(
                    out=wk_sb[:, 3:6, :],
                    in_=wkT[384:768, :].rearrange("(db p) e -> p db e", p=128))
                # small constants after the critical-path loads
                ones_sb = P.tile([128, 128], BF16, tag="ones")
                nc.sync.dma_start(out=ones_sb, in_=ones[:, :])
                bq_sb = vec_sb("bq", bq[:])
                bk_sb = vec_sb("bk", bk[:])
                bv_sb = vec_sb("bv", bv[:])
                bg1_sb = vec_sb("bg1", bg1a[:])
                bg2_sb = vec_sb("bg2", bg2[:])      # host passes bg2/2
                ts_sb = vec_sb("ts", ts[0, :])      # host passes ts/2

                def proj_idx(i):
                    xT = xTs.pop(i)
                    kind, c = divmod(i, NCH)
                    if kind == 0:
                        proj_T(xT, wk_sb, kT[:, :, c * CH:(c + 1) * CH], bk_sb)
                    elif kind == 1:
                        proj_v(xT, wv_sb, c)
                    else:
                        qb = P.tile([128, EB, CH], F32R, tag="qbuf", bufs=3,
                                    name=f"qbuf{c}")
                        proj_T(xT, wq_sb, qb, bq_sb)
                        qbufs[c] = qb

                trans_chunk(0)
                trans_chunk(1)
                for i in range(2 * NCH + 2):         # key, value, q0, q1
                    proj_idx(i)
                    if i + 2 < 2 * NCH + 4:          # transposes up to q3
                        trans_chunk(i + 2)
                    if i == 3:
                        load_w(wv_sb, wvT)
                    elif i == 11:
                        load_w(wq_sb, wqT)
                    elif i == 14:
                        load_w(wg1_sb, wg1T)
                    elif i == 15:
                        load_w(wg2_sb, wg2T)

            # ---- phase C: attention + gate, software-pipelined over i-chunks
            with tc.tile_pool(name="phc", bufs=1) as CP:
                attnT = CP.tile([128, SB, CH], BF16, tag="attnT")
                attTb = CP.tile([128, EB * CH], BF16, tag="attTb")   # gate path
                attTf = CP.tile([128, EB * CH], F32R, tag="attTf")   # output path
                hT = CP.tile([128, EB * CH], BF16, tag="hT")
                g2 = CP.tile([128, EB * CH], BF16, tag="g2")         # tanh(gate/2)
                g3 = CP.tile([128, EB * CH], F32, tag="g3")          # tanh(s1/2)
                gated = CP.tile([128, EB * CH], F32R, tag="gated")
                avs = [None] * NCH            # (att+bv)*ts/2, double-buffered
                                              # across iterations for the tail

                def tail_math(j, sl):
                    """out = (1 + tanh(.25*g2 + .25)) * av on a column slice.

                    s1 = sigmoid(gate) = .5 + .5*g2 ; s2 = sigmoid(s1)
                    s2*att*ts = (1 + tanh(.25*g2 + .25)) * (att+bv)*ts/2
                    """
                    nc.scalar.activation(
                        g3[:, sl], g2[:, sl], AF.Tanh, bias=c25_sb, scale=0.25)
                    nc.vector.scalar_tensor_tensor(
                        gated[:, sl], g3[:, sl], 1.0, avs[j][:, sl],
                        ALU.add, ALU.mult)

                def tail_out(j):
                    """output transposes + store for iteration j (half-major
                    so the last iteration's first halves drain early)."""
                    gv = gated.rearrange("p (eb i) -> p eb i", eb=EB)
                    osbs = [CP.tile([128, D], F32, tag="osb", bufs=2,
                                    name="osb") for _ in range(2)]
                    for half in range(2):
                        cs = slice(half * 384, (half + 1) * 384)
                        for ib in range(2):
                            po = PSC.tile([128, 3, 128], F32R, tag="sc")
                            for k3 in range(3):
                                eb = half * 3 + k3
                                nc.tensor.transpose(
                                    po[:, k3, :],
                                    gv[:, eb, ib * 128:(ib + 1) * 128], ident_sb)
                            nc.vector.tensor_copy(osbs[ib][:, cs], po)
                            r0 = (j * 2 + ib) * 128
                            nc.sync.dma_start(
                                out=out[r0:r0 + 128, cs], in_=osbs[ib][:, cs])

                for ic in range(NCH):
                    qb = qbufs[ic]
                    last = ic == NCH - 1
                    # scores^T + exp per j-block, with the softmax-denominator
                    # chain (ones^T @ exp) interleaved one block behind so the
                    # PE never waits on the last exp
                    dn = PDN.tile([128, CH], F32, tag="dn")
                    for jb in range(SB):
                        ps = PSC.tile([128, CH], F32, tag="sc")
                        for eb in range(EB):
                            nc.tensor.matmul(
                                ps, kT[:, eb, jb * 128:(jb + 1) * 128],
                                qb[:, eb, :],
                                start=(eb == 0), stop=(eb == EB - 1))
                        nc.scalar.activation(
                            attnT[:, jb, :], ps, AF.Exp, scale=SCALE)
                        if jb >= 2:
                            nc.tensor.matmul(
                                dn, ones_sb, attnT[:, jb - 2, :],
                                start=(jb == 2), stop=False)
                    for jb in (SB - 2, SB - 1):
                        nc.tensor.matmul(
                            dn, ones_sb, attnT[:, jb, :],
                            start=False, stop=(jb == SB - 1))
                    recip = CP.tile([128, CH], F32, tag="recip", bufs=1,
                                    name="recip")
                    nc.vector.reciprocal(recip, dn)
                    # next q chunk rides behind the scores
                    if ic + 2 < NCH:
                        proj_idx(2 * NCH + ic + 2)
                        if 2 * NCH + ic + 4 < len(order):
                            trans_chunk(2 * NCH + ic + 4)
                    # attended^T; normalize on DVE; (att+bv)*ts/2 on Pool
                    av = CP.tile([128, EB * CH], F32R, tag="av", bufs=2,
                                 name=f"av{ic}")
                    avs[ic] = av
                    for eb in range(EB):
                        pa = PMM.tile([128, CH], F32, tag="mm")
                        for jb in range(SB):
                            nc.tensor.matmul(
                                pa, v_sb[:, jb, eb * 128:(eb + 1) * 128],
                                attnT[:, jb, :],
                                start=(jb == 0), stop=(jb == SB - 1))
                        sl = slice(eb * CH, (eb + 1) * CH)
                        nc.vector.tensor_mul(attTb[:, sl], pa, recip)
                        nc.vector.tensor_mul(attTf[:, sl], pa, recip)
                        nc.gpsimd.tensor_scalar(
                            av[:, sl], attTf[:, sl], bv_sb[:, eb:eb + 1],
                            ts_sb[:, eb:eb + 1], ALU.add, ALU.mult)
                    # previous iteration's tail fills the attT-eviction wait
                    if ic > 0:
                        tail_math(ic - 1, slice(0, EB * CH))
                        tail_out(ic - 1)
                    # gate1: h = relu(Wg1 att + bg1')
                    for e2 in range(EB):
                        ph = PPG.tile([128, CH], F32, tag="pg")
                        for eb in range(EB):
                            nc.tensor.matmul(
                                ph, wg1_sb[:, eb, e2 * 128:(e2 + 1) * 128],
                                attTb[:, eb * CH:(eb + 1) * CH],
                                start=(eb == 0), stop=(eb == EB - 1))
                        nc.scalar.activation(
                            hT[:, e2 * CH:(e2 + 1) * CH], ph, AF.Relu,
                            bias=bg1_sb[:, e2:e2 + 1])
                    # gate2: g2 = tanh((Wg2 h + bg2)/2); fine-grained tail on
                    # the last iteration so the drain chain is short
                    for e2 in range(EB):
                        pg = PPG.tile([128, CH], F32, tag="pg")
                        for eb in range(EB):
                            nc.tensor.matmul(
                                pg, wg2_sb[:, eb, e2 * 128:(e2 + 1) * 128],
                                hT[:, eb * CH:(eb + 1) * CH],
                                start=(eb == 0), stop=(eb == EB - 1))
                        sl = slice(e2 * CH, (e2 + 1) * CH)
                        nc.scalar.activation(
                            g2[:, sl], pg, AF.Tanh,
                            bias=bg2_sb[:, e2:e2 + 1], scale=0.5)
                        if last:
                            tail_math(ic, sl)
                if NCH:
                    tail_out(NCH - 1)

    nc.compile()
    return nc


def kernel(**inputs):
    if "nc" not in _CACHE:
        _CACHE["nc"] = _build()
    nc = _CACHE["nc"]
    q = np.ascontiguousarray(inputs["query"], dtype=np.float32)
    k = np.ascontiguousarray(inputs["key"], dtype=np.float32)
    vv = np.ascontiguousarray(inputs["value"], dtype=np.float32)
    Wg1 = np.asarray(inputs["Wg1"], np.float32)
    bv_np = np.asarray(inputs["bv"], np.float32)
    bg1a = np.asarray(inputs["bg1"], np.float32) + Wg1 @ bv_np
    shared = {
        "wqT": np.ascontiguousarray(np.asarray(inputs["Wq"], np.float32).T),
        "wkT": np.ascontiguousarray(np.asarray(inputs["Wk"], np.float32).T),
        "wvT": np.ascontiguousarray(np.asarray(inputs["Wv"], np.float32).T),
        "wg1T": np.ascontiguousarray(
            Wg1.T.astype(ml_dtypes.bfloat16)),
        "wg2T": np.ascontiguousarray(
            np.asarray(inputs["Wg2"], np.float32).T.astype(ml_dtypes.bfloat16)),
        "bq": np.ascontiguousarray(inputs["bq"], np.float32),
        "bk": np.ascontiguousarray(inputs["bk"], np.float32),
        "bv": np.ascontiguousarray(bv_np),
        "bg1a": np.ascontiguousarray(bg1a),
        "bg2": np.ascontiguousarray(
            np.asarray(inputs["bg2"], np.float32) * 0.5),
        "ts": np.ascontiguousarray(
            np.asarray(inputs["text_scale"], np.float32) * 0.5),
        "ident": np.eye(128, dtype=np.float32),
        "ones": np.ones((128, 128), dtype=ml_dtypes.bfloat16),
    }
    in_maps = [
        dict(shared, query=q[b], key=k[b], value=vv[b]) for b in range(B)
    ]
    trace = bool(inputs.get("_trace"))
    r = run_bass_kernel_spmd(nc, in_maps, list(range(B)), trace=trace)
    if trace:
        print("HW exec time:", r.exec_time_ns, "ns")
        _CACHE["last_result"] = r
    return np.stack([r.results[b]["out"] for b in range(B)], axis=0)


if __name__ == "__main__":
    pass
